# revision 1
# baseline (speedup 1.0000x reference)
"""Trainium2 Bass kernel for nn_Attention_50354196578449 (sparse_attention).

Reference computation (per batch b of B=64, N=512, MD=QD=AD=1024):
    tq      = query @ Ws                                   # (B, AD)
    h       = tanh(memory_values @ Wh + tq[:, None, :])    # (B, N, AD)
    logits  = squeeze(h @ v)                               # (B, N)
    weights = masked softmax(logits)                       # (B, N)
    context = einsum("bn,bnd->bd", weights, memory_values) # (B, MD)

Strategy: data-parallel over batch across 8 NeuronCores (8 batches/core).
Per core, fully fused on-chip (~208us HW, vs ~109us pure-matmul roofline):
  - mv is loaded once per batch as fp16 (gpsimd cast-DMA) in natural
    layout [n, md]; mv^T tiles come from xbar DMA-transposes
    (one [128,1024] -> [128,8,128] SBUF->SBUF transpose per n-chunk).
  - h^T tiles [ad=128, n=512] via fp16 matmuls (Wh chunk stationary,
    mv^T chunk moving) accumulated over md in PSUM; ACT applies tanh
    with the per-partition bias tq^T[:, b]. fp16 streams at 1 cyc/row
    with the 2-byte LDWEIGHTS hidden under the 512-row stream.
  - logits accumulate in PSUM via v-chunk (M=1) fp16 matmuls over ad,
    lagging the A-phase by one chunk so tanh latency is hidden.
  - masked softmax on partition 0 (DVE/ACT small ops, fp32).
  - context (batches 0..6): w broadcast across partitions via a rank-1
    PE matmul (ones x w), then per-md-chunk DVE multiply+reduce against
    the resident mv^T tiles — keeps the contraction off the PE, which
    is the bottleneck engine mid-loop. The last batch uses M=1 PE
    matmuls instead (PE is idle at the tail, shorter latency there).
  - Wh/Ws stream as fp32 row-block slices on the two HWDGE rails
    (SP + ACT) with pipelined on-chip fp16 casts; a dummy-matmul warmup
    (pinned first via a PSUM WAW dep + explicit ordering edges) keeps
    the PE HAM clock-gate open while they land.
"""

import sys

sys.path.insert(0, "/opt/trn_rl_repo")

from contextlib import ExitStack

import numpy as np

N_CORES = 8
B = 64
B_LOC = B // N_CORES  # 8 batches per core
N = 512
MD = 1024
QD = 1024
AD = 1024
P = 128
NMD = MD // P  # 8 md chunks
NAD = AD // P  # 8 ad chunks
NQD = QD // P  # 8 qd chunks
NNT = N // P   # 4 n chunks
WARMUP_MMS = 150

_CACHE = {}


def _build_nc():
    import concourse.bass as bass  # noqa: F401
    import concourse.tile as tile
    from concourse import bacc, mybir
    from concourse.masks import make_identity

    F32 = mybir.dt.float32
    F16 = mybir.dt.float16
    I32 = mybir.dt.int32
    AF = mybir.ActivationFunctionType
    OP = mybir.AluOpType
    AX = mybir.AxisListType

    nc = bacc.Bacc("TRN2", target_bir_lowering=False)

    mv_d = nc.dram_tensor("memory_values", (B_LOC, N, MD), F32,
                          kind="ExternalInput")
    mask_d = nc.dram_tensor("mask", (B_LOC, N), I32, kind="ExternalInput")
    query_d = nc.dram_tensor("query", (B_LOC, QD), F32, kind="ExternalInput")
    Wh_d = nc.dram_tensor("Wh", (MD, AD), F32, kind="ExternalInput")
    Ws_d = nc.dram_tensor("Ws", (QD, AD), F32, kind="ExternalInput")
    v_d = nc.dram_tensor("v", (AD, 1), F32, kind="ExternalInput")
    ctx_d = nc.dram_tensor("context", (B_LOC, MD), F32, kind="ExternalOutput")

    with tile.TileContext(nc) as tc, ExitStack() as ctx:
        const = ctx.enter_context(tc.tile_pool(name="const", bufs=1))
        nath_pool = ctx.enter_context(tc.tile_pool(name="nath", bufs=3))
        mvT_pool = ctx.enter_context(tc.tile_pool(name="mvT", bufs=3))
        hT_pool = ctx.enter_context(tc.tile_pool(name="hT", bufs=4))
        small = ctx.enter_context(tc.tile_pool(name="small", bufs=2))
        out_pool = ctx.enter_context(tc.tile_pool(name="outp", bufs=2))
        misc_pool = ctx.enter_context(tc.tile_pool(name="misc", bufs=1))
        stage = ctx.enter_context(tc.tile_pool(name="stage", bufs=16))
        psum_h = ctx.enter_context(
            tc.tile_pool(name="psum_h", bufs=5, space="PSUM"))
        psum_tr = ctx.enter_context(
            tc.tile_pool(name="psum_tr", bufs=1, space="PSUM"))
        psum_sm = ctx.enter_context(
            tc.tile_pool(name="psum_sm", bufs=2, space="PSUM"))

        # ---- identities + PE warmup (keeps HAM at full clock while the
        # ---- prologue DMAs stream in) -------------------------------------
        ident_f = const.tile([P, P], F32)
        make_identity(nc, ident_f[:])
        # fp16 identity straight from gpsimd so the warmup isn't gated on
        # a DVE copy the scheduler may deprioritize
        ident_h = const.tile([P, P], F16)
        make_identity(nc, ident_h[:])

        # Warmup matmuls write into the first A-phase PSUM tile: the WAW
        # dependency pins them before the real chain in the scheduler.
        # Every later PE instruction that waits on prologue DMAs also gets
        # an explicit ordering dep on the last warmup matmul so the
        # scheduler cannot hoist it (and its long wait) in front.
        import bass_rust as _br

        ps_h0 = psum_h.tile([P, N], F32, name="ps_h", tag="ps_h")
        last_warm = None
        for _ in range(WARMUP_MMS):
            last_warm = nc.tensor.matmul(ps_h0[:, 0:P], ident_h[:],
                                         ident_h[:], start=True, stop=True,
                                         skip_group_check=True)

        def after_warmup(bi):
            _br.add_dep_helper(bi.ins, last_warm.ins, sync=False,
                               reason="keep warmup at the head of the PE stream")
            return bi

        # ---- small loads --------------------------------------------------
        qf_sb = misc_pool.tile([B_LOC, QD], F32, tag="qf")
        nc.sync.dma_start(qf_sb[:], query_d[:])
        q_sb = const.tile([B_LOC, QD], F16)
        nc.vector.tensor_copy(q_sb[:], qf_sb[:])
        vf_sb = misc_pool.tile([P, NAD], F32, tag="vf")
        nc.sync.dma_start(vf_sb[:], v_d[:].rearrange("(c p) x -> p (c x)", p=P))
        v_sb = const.tile([P, NAD, 2], F16)
        nc.vector.tensor_copy(v_sb[:, :, 0:1], vf_sb[:].unsqueeze(-1))
        # mask -> fp32 on partition 0 directly (SWDGE cast-DMA)
        maskf = misc_pool.tile([1, B_LOC, N], F32, tag="mf")
        nc.gpsimd.dma_start(maskf[:], mask_d[:])

        # first batch: natural fp16 load (SWDGE cast) + xbar transposes
        naths = [None] * B_LOC
        mvTs = [None] * B_LOC

        def emit_loads(b):
            """fp16 natural-layout load + xbar transposes for batch b."""
            nath = nath_pool.tile([P, NNT, MD], F16, tag="nath")
            trig = nc.gpsimd.dma_start(
                nath[:], mv_d[b].rearrange("(t p) m -> p t m", p=P))
            naths[b] = nath
            mvT = mvT_pool.tile([P, NMD, N], F16, tag="mvT")
            for t in range(NNT):
                nc.sync.dma_start_transpose(
                    mvT[:, :, t * P:(t + 1) * P], nath[:, t, :])
            mvTs[b] = mvT
            return trig

        # Ws row-block slices ride the ACT HWDGE rail, Wh the SP rail,
        # launching in parallel; each 512KB slice is a 128x4KB-descriptor
        # transfer (cheap trigger) cast to fp16 as it lands.
        Ws_sb = const.tile([P, NQD, AD], F16)
        Wh_sb = const.tile([P, NMD, AD], F16)
        Ws_view = Ws_d[:].rearrange("(c p) a -> p c a", p=P)
        Wh_view = Wh_d[:].rearrange("(c p) a -> p c a", p=P)
        # All 16 slice triggers first (no cast op may block a trigger),
        # Wh before Ws (Wh + nath(0) gate the first A-phase group; Ws only
        # gates the first tanh via tq), alternating across the two HWDGE
        # rails. Then the casts, Wh first, alternating DVE/ACT.
        wh_sls, ws_sls = [], []
        for c in range(NMD):
            sl = stage.tile([P, AD], F32, tag="stage", name="wh_sl")
            (nc.sync if c % 2 == 0 else nc.scalar).dma_start(
                sl[:], Wh_view[:, c, :])
            wh_sls.append(sl)
        for c in range(NQD):
            sl = stage.tile([P, AD], F32, tag="stage", name="ws_sl")
            (nc.scalar if c % 2 == 0 else nc.sync).dma_start(
                sl[:], Ws_view[:, c, :])
            ws_sls.append(sl)
        emit_loads(0)
        for c in range(NMD):
            if c % 2 == 0:
                nc.vector.tensor_copy(Wh_sb[:, c, :], wh_sls[c][:])
            else:
                nc.scalar.copy(Wh_sb[:, c, :], wh_sls[c][:])
        for c in range(NQD):
            if c % 2 == 0:
                nc.vector.tensor_copy(Ws_sb[:, c, :], ws_sls[c][:])
            else:
                nc.scalar.copy(Ws_sb[:, c, :], ws_sls[c][:])

        # ---- mask precompute (partition 0) --------------------------------
        mx = const.tile([1, B_LOC, 1], F32)
        nc.vector.reduce_max(mx[:], maskf[:], axis=AX.X)

        # ---- query^T + tq^T = (query @ Ws)^T as [p(ad), adc, b] -----------
        qT_sb = const.tile([P, NQD, B_LOC], F16)
        for c in range(NQD):
            ps_q = psum_tr.tile([P, B_LOC], F16, tag="tr")
            after_warmup(
                nc.tensor.transpose(ps_q[:], q_sb[:, c * P:(c + 1) * P],
                                    ident_h[:B_LOC, :B_LOC]))
            nc.vector.tensor_copy(qT_sb[:, c, :], ps_q[:])
        tqT_sb = const.tile([P, NAD, B_LOC], F32)

        def emit_tq(adc):
            """One tq^T column group; interleaved into batch-0's A-phase."""
            ps_tq = psum_sm.tile([P, B_LOC], F32, tag="sm", name="ps_tq")
            for qc in range(NQD):
                after_warmup(nc.tensor.matmul(
                    ps_tq[:], Ws_sb[:, qc, adc * P:(adc + 1) * P],
                    qT_sb[:, qc, :], start=(qc == 0), stop=(qc == NQD - 1)))
            nc.vector.tensor_copy(tqT_sb[:, adc, :], ps_tq[:])

        ones_h = const.tile([1, P], F16)
        nc.gpsimd.memset(ones_h[:], 1.0)

        wbs = [None] * B_LOC
        ctxs = [None] * B_LOC

        def emit_D_compute(b):
            """context^T[md, b] on DVE: broadcast w(b) across partitions via
            a rank-1 PE matmul (ones x w), then one multiply+reduce per md
            chunk against the resident mv^T tiles."""
            ps_wbc = psum_tr.tile([P, N], F32, tag="tr", name="ps_wbc")
            nc.tensor.matmul(ps_wbc[:], ones_h[:], wbs[b][:],
                             start=True, stop=True, skip_group_check=True)
            wbc = small.tile([P, N], F16, tag="wbc")
            nc.scalar.copy(wbc[:], ps_wbc[:])
            ctx_b = small.tile([P, NMD], F32, tag="ctxb")
            for mdc in range(NMD):
                scr = small.tile([P, N], F16, tag="dscr")
                nc.vector.tensor_mul(scr[:], mvTs[b][:, mdc, :], wbc[:])
                nc.vector.reduce_sum(ctx_b[:, mdc:mdc + 1], scr[:],
                                     axis=AX.X)
            ctxs[b] = ctx_b

        def emit_D_out(b):
            """ctx^T [128(md_lo), 8(mdc)] -> [8, 128] -> DRAM."""
            ps_c = psum_tr.tile([B_LOC, P], F32, tag="tr", name="ps_c")
            nc.tensor.transpose(ps_c[:], ctxs[b][:], ident_f[:P, :P])
            out_sb = out_pool.tile([NMD, P], F32)
            nc.vector.tensor_copy(out_sb[:], ps_c[:])
            nc.sync.dma_start(
                ctx_d[b:b + 1, :].rearrange("x (c p) -> (x c) p", p=P),
                out_sb[:])

        for b in range(B_LOC):
            if b + 1 < B_LOC:
                emit_loads(b + 1)

            mvT = mvTs[b]
            ps_log = psum_sm.tile([1, N], F32, tag="sm")
            hTs = []
            for adc in range(NAD):
                if b == 0 and adc == 0:
                    ps_h = ps_h0
                else:
                    ps_h = psum_h.tile([P, N], F32, name="ps_h", tag="ps_h")
                for mdc in range(NMD):
                    mm = nc.tensor.matmul(
                        ps_h[:], Wh_sb[:, mdc, adc * P:(adc + 1) * P],
                        mvT[:, mdc, :], start=(mdc == 0),
                        stop=(mdc == NMD - 1))
                    if b == 0:
                        after_warmup(mm)
                if b == 0:
                    emit_tq(adc)
                hT = hT_pool.tile([P, N], F16)
                nc.scalar.activation(hT[:], ps_h[:], AF.Tanh,
                                     bias=tqT_sb[:, adc, b:b + 1])
                hTs.append(hT)
                if adc >= 1:
                    nc.tensor.matmul(ps_log[:], v_sb[:, adc - 1, 0:1],
                                     hTs[adc - 1], start=(adc - 1 == 0),
                                     stop=False, skip_group_check=True)
                if b > 0 and adc == 2:
                    emit_D_compute(b - 1)

            # fill the tanh(7) latency with last batch's context output
            if b > 0:
                emit_D_out(b - 1)

            nc.tensor.matmul(ps_log[:], v_sb[:, NAD - 1, 0:1], hTs[NAD - 1],
                             start=False, stop=True, skip_group_check=True)

            # masked softmax on partition 0
            sup = small.tile([1, N], F32, tag="sup")
            nc.vector.tensor_scalar(sup[:], maskf[0:1, b, :], 1e30, -1e30,
                                    op0=OP.mult, op1=OP.add)
            ml = small.tile([1, N], F32, tag="ml")
            nc.vector.scalar_tensor_tensor(
                ml[:], in0=sup[:], scalar=mx[0:1, b, :],
                in1=ps_log[:], op0=OP.mult, op1=OP.add)
            rmx = small.tile([1, 1], F32, tag="rmx")
            nc.vector.reduce_max(rmx[:], ml[:], axis=AX.X)
            nrmx = small.tile([1, 1], F32, tag="nrmx")
            nc.vector.tensor_scalar(nrmx[:], rmx[:], -1.0, None, op0=OP.mult)
            et = small.tile([1, N], F32, tag="et")
            zs = small.tile([1, 1], F32, tag="zs")
            nc.scalar.activation(et[:], ml[:], AF.Exp, bias=nrmx[:],
                                 accum_out=zs[:])
            rz = small.tile([1, 1], F32, tag="rz")
            nc.vector.reciprocal(rz[:], zs[:])
            wb = small.tile([1, N], F16, tag="wb")
            nc.vector.tensor_scalar(wb[:], et[:], rz[:],
                                    mx[0:1, b, :], op0=OP.mult, op1=OP.mult)
            wbs[b] = wb

        # last batch: context on the PE (idle at the tail; its latency there
        # beats the DVE chain)
        b = B_LOC - 1
        ps_wT = psum_sm.tile([P, NNT, 2], F16, tag="sm", name="ps_wT")
        for t in range(NNT):
            nc.tensor.matmul(ps_wT[:, t, 0:1],
                             wbs[b][0:1, t * P:(t + 1) * P],
                             ident_h[0:1, 0:1], is_transpose=True,
                             skip_group_check=True)
        wT = small.tile([P, NNT, 2], F16, tag="wT")
        nc.vector.tensor_copy(wT[:, :, 0:1], ps_wT[:, :, 0:1])
        out_sb = out_pool.tile([1, MD], F32, name="out_sb")
        for md2 in range(2):
            ps_c2 = psum_sm.tile([1, MD // 2], F32, tag="sm", name="ps_c2")
            for t in range(NNT):
                nc.tensor.matmul(
                    ps_c2[:], wT[:, t, 0:1],
                    naths[b][:, t, md2 * 512:(md2 + 1) * 512],
                    start=(t == 0), stop=(t == NNT - 1),
                    skip_group_check=True)
            nc.vector.tensor_copy(out_sb[0:1, md2 * 512:(md2 + 1) * 512],
                                  ps_c2[:])
        nc.sync.dma_start(ctx_d[b:b + 1, :], out_sb[:])

    nc.compile()
    return nc


def _get_nc():
    if "nc" not in _CACHE:
        _CACHE["nc"] = _build_nc()
    return _CACHE["nc"]


def kernel(memory_values, mask, query, Wh, Ws, v):
    from concourse.bass_utils import run_bass_kernel_spmd

    nc = _get_nc()
    memory_values = np.ascontiguousarray(memory_values, dtype=np.float32)
    mask = np.ascontiguousarray(mask, dtype=np.int32)
    query = np.ascontiguousarray(query, dtype=np.float32)
    Wh = np.ascontiguousarray(Wh, dtype=np.float32)
    Ws = np.ascontiguousarray(Ws, dtype=np.float32)
    v = np.ascontiguousarray(v, dtype=np.float32)

    in_maps = []
    for c in range(N_CORES):
        s = slice(c * B_LOC, (c + 1) * B_LOC)
        in_maps.append({
            "memory_values": memory_values[s],
            "mask": mask[s],
            "query": query[s],
            "Wh": Wh,
            "Ws": Ws,
            "v": v,
        })
    res = run_bass_kernel_spmd(nc, in_maps, core_ids=list(range(N_CORES)))
    out = np.concatenate([res.results[c]["context"] for c in range(N_CORES)],
                         axis=0)
    return out.astype(np.float32)



# revision 3
# speedup vs baseline: 1.2322x; 1.2322x over previous
"""Trainium2 Bass kernel for nn_Attention_50354196578449 (sparse_attention).

Reference computation (per batch b of B=64, N=512, MD=QD=AD=1024):
    tq      = query @ Ws                                   # (B, AD)
    h       = tanh(memory_values @ Wh + tq[:, None, :])    # (B, N, AD)
    logits  = squeeze(h @ v)                               # (B, N)
    weights = masked softmax(logits)                       # (B, N)
    context = einsum("bn,bnd->bd", weights, memory_values) # (B, MD)

Strategy: data-parallel over batch across 8 NeuronCores (8 batches/core).
Per core, fully fused on-chip. The big h matmul (96% of FLOPs) runs in
double-FP8 mode (DoubleRow, 2x PE throughput):
  - Host pre-casts: mv/query/Ws/v -> fp16; Wh*32 -> fp8e4 in a
    parity-interleaved layout [p, jc, i, ad] with md = 256*jc + 2*p + i,
    matching what a u16-pair xbar transpose of fp8 data produces.
  - Per batch, mv streams in twice: fp16 natural layout (context path,
    HWDGE) and an fp16->fp8 SWDGE cast load (A-phase path). The fp8
    tile is transposed as packed u16 PAIRS (xbar is 2-byte only), giving
    mvT8[q, jc, t, x] = fp8 pair (md=256jc+2q, +1) at n=t*128+x. A
    strided AP splits the pair dim into the DoubleRow K-chunk dim.
  - A-phase: per ad-chunk, 4 DoubleRow matmuls (256-wide K each)
    accumulate 32*(mv@Wh) in PSUM; ACT applies tanh with scale=1/32 and
    the per-partition bias tq^T[:, b].
  - logits move OFF the PE: DVE accumulates acc += hT_chunk * v[chunk]
    (per-partition scalar MAC, fp16), then one M=1 ones-matmul reduces
    across partitions into PSUM.
  - masked softmax on partition 0 (unchanged).
  - context: PE M=1 matmuls streaming the fp16 natural-layout tile
    (w^T per n-chunk via tiny PE transposes), emitted lagged into the
    next batch's A-phase stream.
  - A short dummy-matmul warmup (pinned first via a PSUM WAW dep +
    explicit ordering edges) keeps the PE HAM clock-gate open while the
    prologue DMAs (Wh8 -> sync rail, Ws halves -> both rails) land.
"""

import sys

sys.path.insert(0, "/opt/trn_rl_repo")

from contextlib import ExitStack

import numpy as np

N_CORES = 8
B = 64
B_LOC = B // N_CORES  # 8 batches per core
N = 512
MD = 1024
QD = 1024
AD = 1024
P = 128
NJC = 4        # 256-wide DoubleRow K groups over md
NAD = AD // P  # 8 ad chunks
NQD = QD // P  # 8 qd chunks
NNT = N // P   # 4 n chunks
WH_SCALE = 32.0
WARMUP_MMS = 90

_CACHE = {}


def _build_nc():
    import concourse.bass as bass  # noqa: F401
    import concourse.tile as tile
    from concourse import bacc, mybir
    from concourse.masks import make_identity

    F32 = mybir.dt.float32
    F16 = mybir.dt.float16
    F8 = mybir.dt.float8e4
    I32 = mybir.dt.int32
    AF = mybir.ActivationFunctionType
    OP = mybir.AluOpType
    AX = mybir.AxisListType
    PM = mybir.MatmulPerfMode.DoubleRow

    nc = bacc.Bacc("TRN2", target_bir_lowering=False)

    mv_d = nc.dram_tensor("memory_values", (B_LOC, N, MD), F16,
                          kind="ExternalInput")
    mask_d = nc.dram_tensor("mask", (B_LOC, N), I32, kind="ExternalInput")
    query_d = nc.dram_tensor("query", (B_LOC, QD), F16, kind="ExternalInput")
    Wh8_d = nc.dram_tensor("Wh8", (P, NJC, 2, AD), F8, kind="ExternalInput")
    Ws_d = nc.dram_tensor("Ws", (QD, AD), F16, kind="ExternalInput")
    v_d = nc.dram_tensor("v", (AD, 1), F32, kind="ExternalInput")
    ctx_d = nc.dram_tensor("context", (B_LOC, MD), F32, kind="ExternalOutput")

    with tile.TileContext(nc) as tc, ExitStack() as ctx:
        const = ctx.enter_context(tc.tile_pool(name="const", bufs=1))
        nath16_pool = ctx.enter_context(tc.tile_pool(name="nath16", bufs=3))
        nath8_pool = ctx.enter_context(tc.tile_pool(name="nath8", bufs=2))
        mvT_pool = ctx.enter_context(tc.tile_pool(name="mvT", bufs=3))
        hT_pool = ctx.enter_context(tc.tile_pool(name="hT", bufs=4))
        acc_pool = ctx.enter_context(tc.tile_pool(name="acc", bufs=3))
        small = ctx.enter_context(tc.tile_pool(name="small", bufs=2))
        out_pool = ctx.enter_context(tc.tile_pool(name="outp", bufs=2))
        misc_pool = ctx.enter_context(tc.tile_pool(name="misc", bufs=1))
        psum_h = ctx.enter_context(
            tc.tile_pool(name="psum_h", bufs=4, space="PSUM"))
        psum_tr = ctx.enter_context(
            tc.tile_pool(name="psum_tr", bufs=2, space="PSUM"))
        psum_sm = ctx.enter_context(
            tc.tile_pool(name="psum_sm", bufs=2, space="PSUM"))

        # ---- identities + PE warmup (keeps HAM at full clock while the
        # ---- prologue DMAs stream in) -------------------------------------
        ident_h = const.tile([P, P], F16)
        make_identity(nc, ident_h[:])

        import bass_rust as _br

        ps_h0 = psum_h.tile([P, N], F32, name="ps_h", tag="ps_h")
        last_warm = None
        for _ in range(WARMUP_MMS):
            last_warm = nc.tensor.matmul(ps_h0[:, 0:P], ident_h[:],
                                         ident_h[:], start=True, stop=True,
                                         skip_group_check=True)

        def after_warmup(bi):
            _br.add_dep_helper(bi.ins, last_warm.ins, sync=False,
                               reason="keep warmup at the head of the PE stream")
            return bi

        # ---- prologue loads ------------------------------------------------
        # sync rail: Wh8 (1MB, gates the A-phase) then Ws second half.
        # scalar rail: Ws first half, then small stuff.
        Wh8_sb = const.tile([P, NJC, 2, AD], F8)
        nc.sync.dma_start(Wh8_sb[:], Wh8_d[:])
        Ws_sb = const.tile([P, NQD, AD], F16)
        Ws_view = Ws_d[:].rearrange("(c p) a -> p c a", p=P)
        for c in range(NQD):
            (nc.scalar if c < NQD // 2 else nc.sync).dma_start(
                Ws_sb[:, c, :], Ws_view[:, c, :])
        q_sb = const.tile([B_LOC, QD], F16)
        nc.scalar.dma_start(q_sb[:], query_d[:])
        v_sb = const.tile([P, NAD], F32)
        nc.scalar.dma_start(v_sb[:], v_d[:].rearrange("(c p) x -> p (c x)", p=P))
        # mask -> fp32 on partition 0 directly (SWDGE cast-DMA)
        maskf = misc_pool.tile([1, B_LOC, N], F32, tag="mf")
        nc.gpsimd.dma_start(maskf[:], mask_d[:])

        naths16 = [None] * B_LOC
        naths8 = [None] * B_LOC
        mvTs = [None] * B_LOC

        def emit_loads(b):
            """fp8 SWDGE cast load + packed-u16 xbar transposes, plus the
            fp16 natural-layout load (context path) for batch b."""
            nath8 = nath8_pool.tile([P, NNT, MD], F8, tag="nath8")
            nc.gpsimd.dma_start(
                nath8[:], mv_d[b].rearrange("(t p) m -> p t m", p=P))
            naths8[b] = nath8
            mvT = mvT_pool.tile([P, NJC, NNT, P], F16, tag="mvT")
            for t in range(NNT):
                nc.sync.dma_start_transpose(
                    mvT[:, :, t, :], nath8[:, t, :].bitcast(F16))
            mvTs[b] = mvT
            nath16 = nath16_pool.tile([P, NNT, MD], F16, tag="nath16")
            (nc.scalar if b % 2 == 0 else nc.sync).dma_start(
                nath16[:], mv_d[b].rearrange("(t p) m -> p t m", p=P))
            naths16[b] = nath16

        def mv_rhs(b, jc):
            """DoubleRow moving operand [p, 2(par), 512(t,x)] for K group jc."""
            return (mvTs[b][:, jc, :, :].bitcast(F8)
                    .rearrange("p t (x par) -> p par (t x)", par=2))

        emit_loads(0)

        # ---- mask precompute (partition 0) --------------------------------
        mx = const.tile([1, B_LOC, 1], F32)
        nc.vector.reduce_max(mx[:], maskf[:], axis=AX.X)

        # ---- query^T + tq^T = (query @ Ws)^T as [p(ad), adc, b] -----------
        qT_sb = const.tile([P, NQD, B_LOC], F16)
        for c in range(NQD):
            ps_q = psum_tr.tile([P, B_LOC], F16, tag="tr")
            after_warmup(
                nc.tensor.transpose(ps_q[:], q_sb[:, c * P:(c + 1) * P],
                                    ident_h[:B_LOC, :B_LOC]))
            nc.vector.tensor_copy(qT_sb[:, c, :], ps_q[:])
        tqT_sb = const.tile([P, NAD, B_LOC], F32)

        def emit_tq(adc):
            """One tq^T column group; interleaved into batch-0's A-phase."""
            ps_tq = psum_sm.tile([P, B_LOC], F32, tag="sm", name="ps_tq")
            for qc in range(NQD):
                after_warmup(nc.tensor.matmul(
                    ps_tq[:], Ws_sb[:, qc, adc * P:(adc + 1) * P],
                    qT_sb[:, qc, :], start=(qc == 0), stop=(qc == NQD - 1)))
            nc.vector.tensor_copy(tqT_sb[:, adc, :], ps_tq[:])

        ones_h = const.tile([P, 2], F16)
        nc.gpsimd.memset(ones_h[:], 1.0)

        wbs = [None] * B_LOC
        accs = [None] * B_LOC
        ps_lgs = [None] * B_LOC

        def emit_logred(b):
            """Cross-partition reduce of the DVE h*v accumulator: one M=1
            ones-matmul into PSUM."""
            ps_lg = psum_sm.tile([1, N], F32, tag="sm", name="ps_lg")
            nc.tensor.matmul(ps_lg[:], ones_h[:, 0:1], accs[b][:],
                             start=True, stop=True, skip_group_check=True)
            ps_lgs[b] = ps_lg

        def emit_softmax(b):
            """Masked softmax on partition 0 from ps_lgs[b]."""
            sup = small.tile([1, N], F32, tag="sup")
            nc.vector.tensor_scalar(sup[:], maskf[0:1, b, :], 1e30, -1e30,
                                    op0=OP.mult, op1=OP.add)
            ml = small.tile([1, N], F32, tag="ml")
            nc.vector.scalar_tensor_tensor(
                ml[:], in0=sup[:], scalar=mx[0:1, b, :],
                in1=ps_lgs[b][:], op0=OP.mult, op1=OP.add)
            rmx = small.tile([1, 1], F32, tag="rmx")
            nc.vector.reduce_max(rmx[:], ml[:], axis=AX.X)
            nrmx = small.tile([1, 1], F32, tag="nrmx")
            nc.vector.tensor_scalar(nrmx[:], rmx[:], -1.0, None, op0=OP.mult)
            et = small.tile([1, N], F32, tag="et")
            zs = small.tile([1, 1], F32, tag="zs")
            nc.scalar.activation(et[:], ml[:], AF.Exp, bias=nrmx[:],
                                 accum_out=zs[:])
            rz = small.tile([1, 1], F32, tag="rz")
            nc.vector.reciprocal(rz[:], zs[:])
            wb = small.tile([1, N], F16, tag="wb")
            nc.vector.tensor_scalar(wb[:], et[:], rz[:],
                                    mx[0:1, b, :], op0=OP.mult, op1=OP.mult)
            wbs[b] = wb

        def emit_ctx(b):
            """context[b] = w @ mv on the PE: w^T per n-chunk via tiny PE
            transposes, then M=1 matmuls streaming nath16[b]."""
            ps_wT = psum_tr.tile([P, NNT, 2], F16, tag="tr", name="ps_wT")
            for t in range(NNT):
                nc.tensor.matmul(ps_wT[:, t, 0:1],
                                 wbs[b][0:1, t * P:(t + 1) * P],
                                 ident_h[0:1, 0:1], is_transpose=True,
                                 skip_group_check=True)
            wT = small.tile([P, NNT, 2], F16, tag="wT")
            nc.vector.tensor_copy(wT[:, :, 0:1], ps_wT[:, :, 0:1])
            out_sb = out_pool.tile([1, MD], F32, name="out_sb")
            for md2 in range(2):
                ps_c2 = psum_tr.tile([1, MD // 2], F32, tag="tr", name="ps_c2")
                for t in range(NNT):
                    nc.tensor.matmul(
                        ps_c2[:], wT[:, t, 0:1],
                        naths16[b][:, t, md2 * 512:(md2 + 1) * 512],
                        start=(t == 0), stop=(t == NNT - 1),
                        skip_group_check=True)
                nc.vector.tensor_copy(out_sb[0:1, md2 * 512:(md2 + 1) * 512],
                                      ps_c2[:])
            nc.sync.dma_start(ctx_d[b:b + 1, :], out_sb[:])

        for b in range(B_LOC):
            if b + 1 < B_LOC:
                emit_loads(b + 1)

            acc = None
            for adc in range(NAD):
                if b == 0 and adc == 0:
                    ps_h = ps_h0
                else:
                    ps_h = psum_h.tile([P, N], F32, name="ps_h", tag="ps_h")
                for jc in range(NJC):
                    mm = nc.tensor.matmul(
                        ps_h[:], Wh8_sb[:, jc, :, adc * P:(adc + 1) * P],
                        mv_rhs(b, jc), start=(jc == 0),
                        stop=(jc == NJC - 1), perf_mode=PM)
                    if b == 0:
                        after_warmup(mm)
                if b == 0:
                    emit_tq(adc)
                if b > 0:
                    if adc == 1:
                        emit_logred(b - 1)
                        emit_softmax(b - 1)
                    elif adc == 3:
                        emit_ctx(b - 1)
                hT = hT_pool.tile([P, N], F16)
                nc.scalar.activation(hT[:], ps_h[:], AF.Tanh,
                                     bias=tqT_sb[:, adc, b:b + 1],
                                     scale=1.0 / WH_SCALE)
                acc_new = acc_pool.tile([P, N], F16, tag="acc")
                if adc == 0:
                    nc.vector.tensor_scalar(acc_new[:], hT[:],
                                            v_sb[:, 0:1], None, op0=OP.mult)
                else:
                    nc.vector.scalar_tensor_tensor(
                        acc_new[:], in0=hT[:], scalar=v_sb[:, adc:adc + 1],
                        in1=acc[:], op0=OP.mult, op1=OP.add)
                acc = acc_new
            accs[b] = acc

        # tail: last batch's logits/softmax/context
        b = B_LOC - 1
        emit_logred(b)
        emit_softmax(b)
        emit_ctx(b)

    nc.compile()
    return nc


def _get_nc():
    if "nc" not in _CACHE:
        _CACHE["nc"] = _build_nc()
    return _CACHE["nc"]


def make_in_maps(inputs):
    """Host-side prep: shard over batch, cast to on-chip dtypes, and build
    the parity-interleaved fp8 Wh layout (md = 256*jc + 2*p + i)."""
    import ml_dtypes

    mv = np.ascontiguousarray(inputs["memory_values"], dtype=np.float16)
    mask = np.ascontiguousarray(inputs["mask"], dtype=np.int32)
    query = np.ascontiguousarray(inputs["query"], dtype=np.float16)
    Wh8 = np.ascontiguousarray(
        (np.asarray(inputs["Wh"], dtype=np.float32) * WH_SCALE)
        .astype(ml_dtypes.float8_e4m3)
        .reshape(NJC, P, 2, AD).transpose(1, 0, 2, 3))
    Ws = np.ascontiguousarray(inputs["Ws"], dtype=np.float16)
    v = np.ascontiguousarray(inputs["v"], dtype=np.float32)

    in_maps = []
    for c in range(N_CORES):
        s = slice(c * B_LOC, (c + 1) * B_LOC)
        in_maps.append({
            "memory_values": mv[s],
            "mask": mask[s],
            "query": query[s],
            "Wh8": Wh8,
            "Ws": Ws,
            "v": v,
        })
    return in_maps


def kernel(memory_values, mask, query, Wh, Ws, v):
    from concourse.bass_utils import run_bass_kernel_spmd

    nc = _get_nc()
    in_maps = make_in_maps({
        "memory_values": memory_values, "mask": mask, "query": query,
        "Wh": Wh, "Ws": Ws, "v": v,
    })
    res = run_bass_kernel_spmd(nc, in_maps, core_ids=list(range(N_CORES)))
    out = np.concatenate([res.results[c]["context"] for c in range(N_CORES)],
                         axis=0)
    return out.astype(np.float32)
